# revision 1
# baseline (speedup 1.0000x reference)
"""Trainium2 Bass kernel for nn_DecoderRNN (240-step LSTM decoder, B=512, H=2048).

Sharding: 8-way tensor parallel. Each core owns 1024 of the 8192 gate rows
(256 rows of each of i/f/g/o) and the matching 256 rows of h/c/z. All weights
stay SBUF-resident in bf16. Per step: two group-split AllGathers of h and two
of z (128 rows x 512 batch, bf16); fc2 is computed replicated on every core so
no AllReduce is needed. The one-hot class encoding folds into the gates matmul
as a K=40 tile; biases fold into scalar-engine activations. The emission is
software-pipelined: the next step's W_hh matmuls are emitted between fc1 and
fc2 so the tensor engine has work while the z AllGather is in flight.

Hidden-dim device order is [group, core, row]: device row p = g*1024 + j*128 + r
maps to global hidden row j*256 + g*128 + r (weights are row-permuted on host).
"""

import sys

if "/opt/trn_rl_repo" not in sys.path:
    sys.path.insert(0, "/opt/trn_rl_repo")

import numpy as np
import ml_dtypes

B = 512
OUT = 165
H = 2048
NCLS = 40
NC = 8
BL = B // NC  # batch columns stored per core
KT = H // 128  # 16 k-tiles over the hidden dim
MR = [128, OUT - 128]  # row-tile sizes for the 165-row out/fc2 dim

_CACHE = {}

# Ablation knobs (timing experiments only; ABLATE_CC breaks correctness).
ABLATE_CC = False
ABLATE_STORE = False

# device hidden row p = g*1024 + j*128 + r  <->  global row j*256 + g*128 + r
_g = np.arange(H) // 1024
_j = (np.arange(H) % 1024) // 128
_r = np.arange(H) % 128
PERM = _j * 256 + _g * 128 + _r  # device row p holds global row PERM[p]


def _build(L):
    import concourse.bacc as bacc
    import concourse.mybir as mybir
    import concourse.tile as tile
    from concourse.bass import ds
    from contextlib import ExitStack

    f32 = mybir.dt.float32
    bf16 = mybir.dt.bfloat16
    AF = mybir.ActivationFunctionType
    RG = [list(range(NC))]

    nc = bacc.Bacc("TRN2", target_bir_lowering=False, debug=False, num_devices=NC)

    whh_d = nc.dram_tensor("whh", [H, 1024], bf16, kind="ExternalInput")
    wih_d = nc.dram_tensor("wih", [OUT, 1024], bf16, kind="ExternalInput")
    moh_d = nc.dram_tensor("moh", [NCLS, 1024], bf16, kind="ExternalInput")
    wfc1_d = nc.dram_tensor("wfc1", [H, 256], bf16, kind="ExternalInput")
    wfc2_d = nc.dram_tensor("wfc2", [H, OUT], bf16, kind="ExternalInput")
    onehot_d = nc.dram_tensor("onehot", [NCLS, B], bf16, kind="ExternalInput")
    bgates_d = nc.dram_tensor("bgates", [128, 8], f32, kind="ExternalInput")
    bz_d = nc.dram_tensor("bz", [128, 2], f32, kind="ExternalInput")
    bo_d = nc.dram_tensor("bo", [128, 2], f32, kind="ExternalInput")
    h0_d = nc.dram_tensor("h0", [H, B], bf16, kind="ExternalInput")
    c0_d = nc.dram_tensor("c0", [256, B], f32, kind="ExternalInput")
    out0_d = nc.dram_tensor("out0", [OUT, B], bf16, kind="ExternalInput")
    outs_d = nc.dram_tensor("outs", [L, OUT, BL], f32, kind="ExternalOutput")

    with tile.TileContext(nc) as tc, ExitStack() as ctx:
        const = ctx.enter_context(tc.tile_pool(name="const", bufs=1))
        state = ctx.enter_context(tc.tile_pool(name="state", bufs=2))
        work = ctx.enter_context(tc.tile_pool(name="work", bufs=2))
        psum = ctx.enter_context(tc.tile_pool(name="psum", bufs=8, space="PSUM"))
        dram = ctx.enter_context(tc.tile_pool(name="dram", bufs=3, space="DRAM"))

        pid = nc.gpsimd.partition_id()
        col0 = pid * BL

        # ---- constants into SBUF
        whh_sb = const.tile([128, KT * 1024], bf16, name="whh_sb")
        nc.sync.dma_start(
            whh_sb.rearrange("p (k m) -> p k m", k=KT),
            whh_d.ap().rearrange("(k p) m -> p k m", p=128),
        )
        wih0_sb = const.tile([128, 1024], bf16, name="wih0_sb")
        nc.sync.dma_start(wih0_sb[:], wih_d.ap()[0:128, :])
        wih1_sb = const.tile([37, 1024], bf16, name="wih1_sb")
        nc.sync.dma_start(wih1_sb[:], wih_d.ap()[128:165, :])
        moh_sb = const.tile([NCLS, 1024], bf16, name="moh_sb")
        nc.sync.dma_start(moh_sb[:], moh_d.ap()[:, :])
        wfc1_sb = const.tile([128, KT * 256], bf16, name="wfc1_sb")
        nc.sync.dma_start(
            wfc1_sb.rearrange("p (k m) -> p k m", k=KT),
            wfc1_d.ap().rearrange("(k p) m -> p k m", p=128),
        )
        wfc2_sb = const.tile([128, KT * OUT], bf16, name="wfc2_sb")
        nc.sync.dma_start(
            wfc2_sb.rearrange("p (k m) -> p k m", k=KT),
            wfc2_d.ap().rearrange("(k p) m -> p k m", p=128),
        )
        onehot_sb = const.tile([NCLS, B], bf16, name="onehot_sb")
        nc.sync.dma_start(onehot_sb[:], onehot_d.ap()[:, :])
        bg_sb = const.tile([128, 8], f32, name="bg_sb")
        nc.sync.dma_start(bg_sb[:], bgates_d.ap()[:, :])
        bz_sb = const.tile([128, 2], f32, name="bz_sb")
        nc.sync.dma_start(bz_sb[:], bz_d.ap()[:, :])
        bo_sb = const.tile([128, 2], f32, name="bo_sb")
        nc.sync.dma_start(bo_sb[:], bo_d.ap()[:, :])

        def load_half(dst, src_ap, split=False, eng=None):
            # dst: SBUF [128, 8*B]; src: DRAM [1024, B] (8 row-blocks of 128)
            eng = eng or nc.sync
            if split:
                # first two k-slices land early so dependent matmuls start sooner
                eng.dma_start(
                    dst[:, :2 * B].rearrange("p (k n) -> p k n", k=2),
                    src_ap[0:256, :].rearrange("(k p) n -> p k n", p=128),
                )
                eng.dma_start(
                    dst[:, 2 * B:].rearrange("p (k n) -> p k n", k=6),
                    src_ap[256:1024, :].rearrange("(k p) n -> p k n", p=128),
                )
            else:
                eng.dma_start(
                    dst.rearrange("p (k n) -> p k n", k=8),
                    src_ap.rearrange("(k p) n -> p k n", p=128),
                )

        # ---- initial state (h0 pre-permuted on host to device order)
        hcat = []
        for g in range(2):
            hg = state.tile([128, 8 * B], bf16, tag=f"hcat{g}", name=f"hcat{g}_init")
            load_half(hg, h0_d.ap()[g * 1024:(g + 1) * 1024, :])
            hcat.append(hg)
        outb0 = state.tile([128, B], bf16, tag="outb0", name="outb0_init")
        nc.sync.dma_start(outb0[:], out0_d.ap()[0:128, :])
        outb1 = state.tile([37, B], bf16, tag="outb1", name="outb1_init")
        nc.sync.dma_start(outb1[:], out0_d.ap()[128:165, :])
        c_prev = []
        for g in range(2):
            ct = state.tile([128, B], f32, tag=f"c{g}", name=f"c{g}_init")
            nc.sync.dma_start(ct[:], c0_d.ap()[g * 128:(g + 1) * 128, :])
            c_prev.append(ct)

        def emit_whh(t, mtiles, hc):
            # gates(t) W_hh k-tiles for the given output m-tiles; returns psum tiles
            tiles = {}
            for mt in mtiles:
                ps = psum.tile([128, B], f32, tag="ps", name=f"psg_{t}_{mt}")
                tiles[mt] = ps
                for half in range(2):
                    for kk in range(8):
                        ki = half * 8 + kk
                        nc.tensor.matmul(
                            ps[:],
                            whh_sb[:, ki * 1024 + mt * 128: ki * 1024 + (mt + 1) * 128],
                            hc[half][:, kk * B:(kk + 1) * B],
                            start=(ki == 0),
                            stop=False,
                        )
            return tiles

        def ag(inp, tag, t):
            out_t = dram.tile([1024, B], bf16, tag=tag, name=f"{tag}_{t}",
                              addr_space="Shared")
            if ABLATE_CC:
                nc.sync.dma_start(out_t[0:128, :], inp[:])
            else:
                nc.gpsimd.collective_compute(
                    "AllGather", mybir.AluOpType.bypass, replica_groups=RG,
                    ins=[inp.opt()], outs=[out_t.opt()],
                )
            return out_t

        # prime: gates(0) W_hh for all 8 m-tiles
        psg = emit_whh(0, range(8), hcat)

        for t in range(L):
            # ---- gates(t) tail + LSTM, interleaved per group so the g0
            # AllGather is issued while PE still runs the g1 tail matmuls
            hb_out = [None, None]
            c_new_list = []
            for g in range(2):
                for mt in range(4 * g, 4 * g + 4):
                    ps = psg[mt]
                    nc.tensor.matmul(ps[:], wih0_sb[:, mt * 128:(mt + 1) * 128], outb0[:],
                                     start=False, stop=False)
                    nc.tensor.matmul(ps[:], wih1_sb[:, mt * 128:(mt + 1) * 128], outb1[:],
                                     start=False, stop=False)
                    nc.tensor.matmul(ps[:], moh_sb[:, mt * 128:(mt + 1) * 128], onehot_sb[:],
                                     start=False, stop=True)
                pi, pf, pg_, po = (psg[g * 4 + q] for q in range(4))
                si = work.tile([128, B], f32, tag="si", name=f"si_{t}_{g}")
                nc.scalar.activation(si[:], pi[:], AF.Sigmoid, bias=bg_sb[:, 4 * g: 4 * g + 1])
                sf = work.tile([128, B], f32, tag="sf", name=f"sf_{t}_{g}")
                nc.scalar.activation(sf[:], pf[:], AF.Sigmoid, bias=bg_sb[:, 4 * g + 1: 4 * g + 2])
                tg = work.tile([128, B], f32, tag="tg", name=f"tg_{t}_{g}")
                nc.scalar.activation(tg[:], pg_[:], AF.Tanh, bias=bg_sb[:, 4 * g + 2: 4 * g + 3])
                so = work.tile([128, B], f32, tag="so", name=f"so_{t}_{g}")
                nc.scalar.activation(so[:], po[:], AF.Sigmoid, bias=bg_sb[:, 4 * g + 3: 4 * g + 4])
                m1 = work.tile([128, B], f32, tag="m1", name=f"m1_{t}_{g}")
                nc.vector.tensor_mul(m1[:], si[:], tg[:])
                m2 = work.tile([128, B], f32, tag="m2", name=f"m2_{t}_{g}")
                nc.vector.tensor_mul(m2[:], sf[:], c_prev[g][:])
                c_new = state.tile([128, B], f32, tag=f"c{g}", name=f"c{g}_{t}")
                nc.vector.tensor_add(c_new[:], m1[:], m2[:])
                th = work.tile([128, B], f32, tag="th", name=f"th_{t}_{g}")
                nc.scalar.activation(th[:], c_new[:], AF.Tanh)
                hn = work.tile([128, B], bf16, tag="hn", name=f"hn_{t}_{g}")
                nc.vector.tensor_mul(hn[:], so[:], th[:])
                hb_in = dram.tile([128, B], bf16, tag=f"hbin{g}", name=f"hbin{g}_{t}")
                nc.sync.dma_start(hb_in[:], hn[:])
                hb_out[g] = ag(hb_in, f"hbout{g}", t)
                c_new_list.append(c_new)
            c_prev = c_new_list

            hcat_new = []
            for g in range(2):
                hg = state.tile([128, 8 * B], bf16, tag=f"hcat{g}", name=f"hcat{g}_{t}")
                load_half(hg, hb_out[g], split=(g == 0),
                          eng=(nc.sync if g == 0 else nc.scalar))
                hcat_new.append(hg)

            # ---- fc1 (own 256 rows) + relu -> z group AllGathers
            psz = []
            for mt in range(2):
                ps = psum.tile([128, B], f32, tag="ps", name=f"psz_{t}_{mt}")
                psz.append(ps)
                for half in range(2):
                    for kk in range(8):
                        ki = half * 8 + kk
                        nc.tensor.matmul(
                            ps[:],
                            wfc1_sb[:, ki * 256 + mt * 128: ki * 256 + (mt + 1) * 128],
                            hcat_new[half][:, kk * B:(kk + 1) * B],
                            start=(ki == 0),
                            stop=(ki == KT - 1),
                        )
            zb_out = [None, None]
            for mt in range(2):
                zb = work.tile([128, B], bf16, tag=f"zb{mt}", name=f"zb_{t}_{mt}")
                nc.scalar.activation(zb[:], psz[mt][:], AF.Relu, bias=bz_sb[:, mt:mt + 1])
                zb_in = dram.tile([128, B], bf16, tag=f"zbin{mt}", name=f"zbin{mt}_{t}")
                nc.sync.dma_start(zb_in[:], zb[:])
                zb_out[mt] = ag(zb_in, f"zbout{mt}", t)
            zcat = []
            for g in range(2):
                zg = state.tile([128, 8 * B], bf16, tag=f"zcat{g}", name=f"zcat{g}_{t}")
                load_half(zg, zb_out[g], eng=(nc.sync if g == 0 else nc.scalar))
                zcat.append(zg)

            # ---- prefetch next step's W_hh matmuls (m-tiles 0-5) to cover z AG
            if t + 1 < L:
                psg_next = emit_whh(t + 1, range(6), hcat_new)

            # ---- fc2 (full 165 rows, replicated) + out store + bf16 copy
            new_outb = []
            for mt in range(2):
                mr = MR[mt]
                ps = psum.tile([128, B], f32, tag="ps", name=f"pso_{t}_{mt}")
                for half in range(2):
                    for kk in range(8):
                        ki = half * 8 + kk
                        nc.tensor.matmul(
                            ps[:mr],
                            wfc2_sb[:, ki * OUT + mt * 128: ki * OUT + mt * 128 + mr],
                            zcat[half][:, kk * B:(kk + 1) * B],
                            start=(ki == 0),
                            stop=(ki == KT - 1),
                        )
                of = work.tile([128, B], f32, tag=f"of{mt}", name=f"of_{t}_{mt}")
                nc.scalar.activation(of[:mr], ps[:mr], AF.Identity, bias=bo_sb[:mr, mt:mt + 1])
                if not ABLATE_STORE:
                    nc.gpsimd.dma_start(
                        outs_d.ap()[t, mt * 128: mt * 128 + mr, :],
                        of[:mr, ds(col0, BL)],
                    )
                ob = state.tile([mr, B], bf16, tag=f"outb{mt}", name=f"outb{mt}_{t}")
                nc.vector.tensor_copy(ob[:], of[:mr])
                new_outb.append(ob)
            outb0, outb1 = new_outb

            # ---- remaining next-step W_hh m-tiles
            if t + 1 < L:
                psg_next.update(emit_whh(t + 1, range(6, 8), hcat_new))
                psg = psg_next
            hcat = hcat_new

    nc.compile()
    return nc


def _prepare_in_maps(inputs):
    bf = ml_dtypes.bfloat16
    f = {k: np.asarray(v) for k, v in inputs.items()}
    W_enc = f["W_enc"].astype(np.float32)
    b_enc = f["b_enc"].astype(np.float32)
    W_ih = f["W_ih"].astype(np.float32)
    b_ih = f["b_ih"].astype(np.float32)
    W_hh = f["W_hh"].astype(np.float32)
    b_hh = f["b_hh"].astype(np.float32)
    W_fc1 = f["W_fc1"].astype(np.float32)
    b_fc1 = f["b_fc1"].astype(np.float32)
    W_fc2 = f["W_fc2"].astype(np.float32)
    b_fc2 = f["b_fc2"].astype(np.float32)
    W_inh = f["W_inh"].astype(np.float32)
    b_inh = f["b_inh"].astype(np.float32)
    W_inc = f["W_inc"].astype(np.float32)
    b_inc = f["b_inc"].astype(np.float32)
    labels = f["labels"].astype(np.int64)
    x = f["inputs"].astype(np.float32)

    frame0 = x.reshape(B, OUT)
    h0 = frame0 @ W_inh.T + b_inh            # [B, H]
    c0 = frame0 @ W_inc.T + b_inc            # [B, H]
    onehot = np.zeros((NCLS, B), np.float32)
    onehot[labels, np.arange(B)] = 1.0
    M1 = W_ih[:, OUT:] @ W_enc               # [4H, NCLS]
    bias_gates = b_ih + b_hh + W_ih[:, OUT:] @ b_enc  # [4H]

    in_maps = []
    for j in range(NC):
        mt = np.arange(8)
        gt, g = mt % 4, mt // 4
        rows = (gt[:, None] * H + j * 256 + g[:, None] * 128 + np.arange(128)[None, :]).reshape(-1)
        zrows = j * 256 + np.arange(256)
        bg = bias_gates[rows].reshape(8, 128).T.copy()          # [128, 8]
        bzv = b_fc1[zrows].reshape(2, 128).T.copy()             # [128, 2]
        bov = np.zeros((128, 2), np.float32)
        bov[:, 0] = b_fc2[:128]
        bov[:MR[1], 1] = b_fc2[128:]
        in_maps.append({
            # k-rows over the hidden dim are permuted to device order PERM
            "whh": np.ascontiguousarray(W_hh[np.ix_(rows, PERM)].T).astype(bf),
            "wih": np.ascontiguousarray(W_ih[rows, :OUT].T).astype(bf),
            "moh": np.ascontiguousarray(M1[rows].T).astype(bf),
            "wfc1": np.ascontiguousarray(W_fc1[np.ix_(zrows, PERM)].T).astype(bf),
            "wfc2": np.ascontiguousarray(W_fc2[:, PERM].T).astype(bf),
            "onehot": onehot.astype(bf),
            "bgates": bg,
            "bz": bzv,
            "bo": bov,
            "h0": np.ascontiguousarray(h0.T[PERM]).astype(bf),
            "c0": np.ascontiguousarray(c0.T[zrows]).astype(np.float32),
            "out0": np.ascontiguousarray(frame0.T).astype(bf),
        })
    return in_maps


def _get_program(L):
    if L not in _CACHE:
        _CACHE[L] = _build(L)
    return _CACHE[L]


def kernel(**inputs):
    from concourse.bass_utils import run_bass_kernel_spmd

    L = int(np.asarray(inputs["length"]))
    x = np.asarray(inputs["inputs"])
    Bq, J, D = x.shape
    assert (Bq, J * D) == (B, OUT)

    nc = _get_program(L)
    in_maps = _prepare_in_maps(inputs)
    res = run_bass_kernel_spmd(nc, in_maps, core_ids=list(range(NC)))
    # core j returns [L, OUT, BL] covering batch columns j*BL:(j+1)*BL
    full = np.concatenate([res.results[j]["outs"] for j in range(NC)], axis=2)
    out = np.transpose(full, (2, 0, 1)).reshape(B, L, J, D).astype(np.float32)
    return out



# revision 8
# speedup vs baseline: 1.1414x; 1.1414x over previous
"""Trainium2 Bass kernel for nn_DecoderRNN (240-step LSTM decoder, B=512, H=2048).

Sharding: 8-way tensor parallel on the hidden/gate dims, with the batch split
into two halves (256 columns each) that are processed as two software-pipelined
streams. While half A waits on its collectives (h AllGather after the LSTM
cell, out AllReduce after fc2), the tensor engine runs half B's matmuls, so the
PE never idles long enough for the HAM clock gate to re-throttle it (the
previous version ran every matmul at the cold 1.2 GHz clock and still idled
49% of the time).

Per core: 1024 gate rows (128-row tiles of i/f/g/o x 2 groups), 256 h rows,
256 z rows. fc2 is k-sharded: each core computes W_fc2[:, own z rows] @ z_own
and an AllReduce produces out (replacing the z AllGather + replicated fc2).
Each core's h rows are globally contiguous [j*256,(j+1)*256), so the rank-major
AllGather output is already in natural hidden order - no permutation anywhere.

Block for step t, half X (hcat = gathered h(t)):
  fc1 -> relu -> fc2 partial -> AllReduce out(t-1)   (out AR in flight ...)
  W_hh k-chains -> gates(t) psum (4 banks, 2 m-tiles packed per bank)
  load AR result, cast out(t-1) to bf16, store outs[t-1]
  gates(t) += W_ih @ out(t-1) + M1 @ onehot  (tail)
  LSTM cell -> h(t+1) -> DMA -> AllGather -> load hcat for next block
Block 0 skips the fc1/out part (out(-1) = frame0); an epilogue computes
out(L-1) only.
"""

import sys

if "/opt/trn_rl_repo" not in sys.path:
    sys.path.insert(0, "/opt/trn_rl_repo")

import numpy as np
import ml_dtypes

B = 512
HB = 256        # batch columns per half
OUT = 165
H = 2048
NCLS = 40
NC = 8
BL = B // NC    # output batch columns owned per core
KT = H // 128   # 16 k-tiles over the hidden dim
MR = [128, OUT - 128]

_CACHE = {}


def _build(L):
    import concourse.bacc as bacc
    import concourse.mybir as mybir
    import concourse.tile as tile
    from concourse.bass import ds
    from contextlib import ExitStack

    f32 = mybir.dt.float32
    bf16 = mybir.dt.bfloat16
    AF = mybir.ActivationFunctionType
    RG = [list(range(NC))]

    nc = bacc.Bacc("TRN2", target_bir_lowering=False, debug=False, num_devices=NC)

    whh_d = nc.dram_tensor("whh", [H, 1024], bf16, kind="ExternalInput")
    wih_d = nc.dram_tensor("wih", [OUT, 1024], bf16, kind="ExternalInput")
    moh_d = nc.dram_tensor("moh", [NCLS, 1024], bf16, kind="ExternalInput")
    wfc1_d = nc.dram_tensor("wfc1", [H, 256], bf16, kind="ExternalInput")
    wfc2_d = nc.dram_tensor("wfc2", [256, OUT], bf16, kind="ExternalInput")
    onehot_d = nc.dram_tensor("onehot", [NCLS, B], bf16, kind="ExternalInput")
    bgates_d = nc.dram_tensor("bgates", [128, 8], f32, kind="ExternalInput")
    bz_d = nc.dram_tensor("bz", [128, 2], f32, kind="ExternalInput")
    bo_d = nc.dram_tensor("bo", [128, 2], f32, kind="ExternalInput")
    h0_d = nc.dram_tensor("h0", [H, B], bf16, kind="ExternalInput")
    c0_d = nc.dram_tensor("c0", [256, B], f32, kind="ExternalInput")
    out0_d = nc.dram_tensor("out0", [OUT, B], bf16, kind="ExternalInput")
    # cols 0:BL always hold this core's batch slice; cols BL:2*BL are scratch
    # written by the other half's store (keeps the program identical per core).
    outs_d = nc.dram_tensor("outs", [L, OUT, 2 * BL], f32, kind="ExternalOutput")

    with tile.TileContext(nc) as tc, ExitStack() as ctx:
        const = ctx.enter_context(tc.tile_pool(name="const", bufs=1))
        state = ctx.enter_context(tc.tile_pool(name="state", bufs=2))
        work = ctx.enter_context(tc.tile_pool(name="work", bufs=2))
        psum = ctx.enter_context(tc.tile_pool(name="psum", bufs=8, space="PSUM"))
        dram = ctx.enter_context(tc.tile_pool(name="dram", bufs=3, space="DRAM"))

        pid = nc.gpsimd.partition_id()
        own_half = pid // 4          # which batch half holds this core's columns
        other_half = (pid // 4 + 1) % 2
        csrc = pid % 4 * BL          # column offset of our slice inside that half
        # store dst: the block whose half == own_half writes cols [0,BL)
        dst_off = [own_half * BL, other_half * BL]

        # ---- constants into SBUF
        whh_sb = const.tile([128, KT * 1024], bf16, name="whh_sb")
        nc.sync.dma_start(
            whh_sb.rearrange("p (k m) -> p k m", k=KT),
            whh_d.ap().rearrange("(k p) m -> p k m", p=128),
        )
        wih0_sb = const.tile([128, 1024], bf16, name="wih0_sb")
        nc.sync.dma_start(wih0_sb[:], wih_d.ap()[0:128, :])
        wih1_sb = const.tile([37, 1024], bf16, name="wih1_sb")
        nc.sync.dma_start(wih1_sb[:], wih_d.ap()[128:165, :])
        moh_sb = const.tile([NCLS, 1024], bf16, name="moh_sb")
        nc.sync.dma_start(moh_sb[:], moh_d.ap()[:, :])
        wfc1_sb = const.tile([128, KT * 256], bf16, name="wfc1_sb")
        nc.sync.dma_start(
            wfc1_sb.rearrange("p (k m) -> p k m", k=KT),
            wfc1_d.ap().rearrange("(k p) m -> p k m", p=128),
        )
        wfc2_sb = const.tile([128, 2 * OUT], bf16, name="wfc2_sb")
        nc.sync.dma_start(
            wfc2_sb.rearrange("p (k m) -> p k m", k=2),
            wfc2_d.ap().rearrange("(k p) m -> p k m", p=128),
        )
        onehot_sb = const.tile([NCLS, B], bf16, name="onehot_sb")
        nc.sync.dma_start(onehot_sb[:], onehot_d.ap()[:, :])
        bg_sb = const.tile([128, 8], f32, name="bg_sb")
        nc.sync.dma_start(bg_sb[:], bgates_d.ap()[:, :])
        bz_sb = const.tile([128, 2], f32, name="bz_sb")
        nc.sync.dma_start(bz_sb[:], bz_d.ap()[:, :])
        bo_sb = const.tile([128, 2], f32, name="bo_sb")
        nc.sync.dma_start(bo_sb[:], bo_d.ap()[:, :])

        # ---- initial state per half: hcat, c, outb
        hcat = [None, None]
        c_prev = [[None, None], [None, None]]
        outb0 = [None, None]
        outb1 = [None, None]
        for X in range(2):
            hg = state.tile([128, KT * HB], bf16, tag=f"hcat{X}", name=f"hcat{X}_init")
            nc.scalar.dma_start(
                hg.rearrange("p (k n) -> p k n", k=KT),
                h0_d.ap()[:, X * HB:(X + 1) * HB].rearrange("(k p) n -> p k n", p=128),
            )
            hcat[X] = hg
            for g in range(2):
                ct = state.tile([128, HB], f32, tag=f"c{X}{g}", name=f"c{X}{g}_init")
                nc.sync.dma_start(ct[:], c0_d.ap()[g * 128:(g + 1) * 128, X * HB:(X + 1) * HB])
                c_prev[X][g] = ct
            ob0 = state.tile([128, HB], bf16, tag=f"outb0{X}", name=f"outb0{X}_init")
            nc.sync.dma_start(ob0[:], out0_d.ap()[0:128, X * HB:(X + 1) * HB])
            outb0[X] = ob0
            ob1 = state.tile([37, HB], bf16, tag=f"outb1{X}", name=f"outb1{X}_init")
            nc.sync.dma_start(ob1[:], out0_d.ap()[128:165, X * HB:(X + 1) * HB])
            outb1[X] = ob1

        def emit_fc_out(t, X, hc):
            """fc1 -> relu -> fc2 partial -> AllReduce; returns ar_out dram tile.
            Produces out(t-1) for step t's tail (and outs[t-1] store)."""
            # one start/stop per bank: start clears has_written for the WHOLE
            # 2KB bank; each region's first start=False matmul overwrites
            # (bit clear) and later ones accumulate.
            ps_z = psum.tile([128, 512], f32, tag="ps", name=f"psz_{t}_{X}")
            for mt in range(2):
                for ki in range(KT):
                    nc.tensor.matmul(
                        ps_z[:, mt * HB:(mt + 1) * HB],
                        wfc1_sb[:, ki * 256 + mt * 128: ki * 256 + (mt + 1) * 128],
                        hc[:, ki * HB:(ki + 1) * HB],
                        start=(mt == 0 and ki == 0),
                        stop=(mt == 1 and ki == KT - 1),
                    )
            zb = work.tile([128, 512], bf16, tag=f"zb{X}", name=f"zb_{t}_{X}")
            for mt in range(2):
                nc.scalar.activation(
                    zb[:, mt * HB:(mt + 1) * HB], ps_z[:, mt * HB:(mt + 1) * HB],
                    AF.Relu, bias=bz_sb[:, mt:mt + 1],
                )
            ps_o = psum.tile([128, 512], f32, tag="ps", name=f"pso_{t}_{X}")
            for mt in range(2):
                mr = MR[mt]
                for ki in range(2):
                    nc.tensor.matmul(
                        ps_o[:mr, mt * HB:mt * HB + HB],
                        wfc2_sb[:, ki * OUT + mt * 128: ki * OUT + mt * 128 + mr],
                        zb[:, ki * HB:(ki + 1) * HB],
                        start=(mt == 0 and ki == 0),
                        stop=(mt == 1 and ki == 1),
                    )
            of = work.tile([128, 512], f32, tag=f"of{X}", name=f"of_{t}_{X}")
            for mt in range(2):
                mr = MR[mt]
                nc.scalar.activation(
                    of[:mr, mt * HB:mt * HB + HB], ps_o[:mr, mt * HB:mt * HB + HB],
                    AF.Identity, bias=bo_sb[:mr, mt:mt + 1],
                )
            ar_in = dram.tile([OUT, HB], f32, tag=f"arin{X}", name=f"arin_{t}_{X}")
            nc.sync.dma_start(ar_in[0:128, :], of[:, 0:HB])
            nc.sync.dma_start(ar_in[128:165, :], of[:37, HB:2 * HB])
            ar_out = dram.tile([OUT, HB], f32, tag=f"arout{X}", name=f"arout_{t}_{X}",
                               addr_space="Shared")
            nc.gpsimd.collective_compute(
                "AllReduce", mybir.AluOpType.add, replica_groups=RG,
                ins=[ar_in.opt()], outs=[ar_out.opt()],
            )
            return ar_out

        def emit_whh(t, X, hc):
            """W_hh k-chains into 4 packed psum banks: bank b = (mt 2b, 2b+1)."""
            gb = []
            for b in range(4):
                ps = psum.tile([128, 512], f32, tag="ps", name=f"psg_{t}_{X}_{b}")
                gb.append(ps)
                for sub in range(2):
                    mt = b * 2 + sub
                    for ki in range(KT):
                        nc.tensor.matmul(
                            ps[:, sub * HB:(sub + 1) * HB],
                            whh_sb[:, ki * 1024 + mt * 128: ki * 1024 + (mt + 1) * 128],
                            hc[:, ki * HB:(ki + 1) * HB],
                            start=(sub == 0 and ki == 0),
                            stop=False,
                        )
            return gb

        def emit_ar_consume(t, X, ar_out, store_t):
            """Load the AllReduce result: cast to bf16 outb tiles + store outs."""
            arl = work.tile([128, 512], f32, tag=f"arl{X}", name=f"arl_{t}_{X}")
            nc.sync.dma_start(arl[:, 0:HB], ar_out[0:128, :])
            nc.sync.dma_start(arl[:37, HB:2 * HB], ar_out[128:165, :])
            ob0 = state.tile([128, HB], bf16, tag=f"outb0{X}", name=f"outb0_{t}_{X}")
            nc.vector.tensor_copy(ob0[:], arl[:, 0:HB])
            ob1 = state.tile([37, HB], bf16, tag=f"outb1{X}", name=f"outb1_{t}_{X}")
            nc.vector.tensor_copy(ob1[:], arl[:37, HB:2 * HB])
            nc.gpsimd.dma_start(
                outs_d.ap()[store_t, 0:128, ds(dst_off[X], BL)],
                arl[:, ds(csrc, BL)],
            )
            nc.gpsimd.dma_start(
                outs_d.ap()[store_t, 128:165, ds(dst_off[X], BL)],
                arl[:37, ds(HB + csrc, BL)],
            )
            return ob0, ob1

        def emit_tail(t, X, gb, ob0, ob1):
            for b in range(4):
                for sub in range(2):
                    mt = b * 2 + sub
                    dst = gb[b][:, sub * HB:(sub + 1) * HB]
                    nc.tensor.matmul(dst, wih0_sb[:, mt * 128:(mt + 1) * 128], ob0[:],
                                     start=False, stop=False)
                    nc.tensor.matmul(dst, wih1_sb[:, mt * 128:(mt + 1) * 128], ob1[:],
                                     start=False, stop=False)
                    nc.tensor.matmul(dst, moh_sb[:, mt * 128:(mt + 1) * 128],
                                     onehot_sb[:, X * HB:(X + 1) * HB],
                                     start=False, stop=(sub == 1))

        def emit_lstm_ag(t, X, gb):
            """LSTM cell from gate banks -> h(t+1) slice -> AllGather; returns
            (hb_out dram tile, new c tiles)."""
            hn = work.tile([128, 512], bf16, tag=f"hn{X}", name=f"hn_{t}_{X}")
            cn_new = [None, None]
            for g in range(2):
                bi, bo_ = gb[2 * g], gb[2 * g + 1]
                si = work.tile([128, HB], f32, tag=f"si{X}", name=f"si_{t}_{X}_{g}")
                nc.scalar.activation(si[:], bi[:, 0:HB], AF.Sigmoid,
                                     bias=bg_sb[:, 4 * g + 0: 4 * g + 1])
                sf = work.tile([128, HB], f32, tag=f"sf{X}", name=f"sf_{t}_{X}_{g}")
                nc.scalar.activation(sf[:], bi[:, HB:2 * HB], AF.Sigmoid,
                                     bias=bg_sb[:, 4 * g + 1: 4 * g + 2])
                tg = work.tile([128, HB], f32, tag=f"tg{X}", name=f"tg_{t}_{X}_{g}")
                nc.scalar.activation(tg[:], bo_[:, 0:HB], AF.Tanh,
                                     bias=bg_sb[:, 4 * g + 2: 4 * g + 3])
                so = work.tile([128, HB], f32, tag=f"so{X}", name=f"so_{t}_{X}_{g}")
                nc.scalar.activation(so[:], bo_[:, HB:2 * HB], AF.Sigmoid,
                                     bias=bg_sb[:, 4 * g + 3: 4 * g + 4])
                m1 = work.tile([128, HB], f32, tag=f"m1{X}", name=f"m1_{t}_{X}_{g}")
                nc.vector.tensor_mul(m1[:], si[:], tg[:])
                m2 = work.tile([128, HB], f32, tag=f"m2{X}", name=f"m2_{t}_{X}_{g}")
                nc.vector.tensor_mul(m2[:], sf[:], c_prev[X][g][:])
                cn = state.tile([128, HB], f32, tag=f"c{X}{g}", name=f"c{X}{g}_{t}")
                nc.vector.tensor_add(cn[:], m1[:], m2[:])
                cn_new[g] = cn
                th = work.tile([128, HB], f32, tag=f"th{X}", name=f"th_{t}_{X}_{g}")
                nc.scalar.activation(th[:], cn[:], AF.Tanh)
                nc.vector.tensor_mul(hn[:, g * HB:(g + 1) * HB], so[:], th[:])
            hb_in = dram.tile([256, HB], bf16, tag=f"hbin{X}", name=f"hbin_{t}_{X}")
            nc.sync.dma_start(
                hb_in.rearrange("(g p) n -> p g n", p=128),
                hn.rearrange("p (g n) -> p g n", g=2),
            )
            hb_out = dram.tile([H, HB], bf16, tag=f"hbout{X}", name=f"hbout_{t}_{X}",
                               addr_space="Shared")
            nc.gpsimd.collective_compute(
                "AllGather", mybir.AluOpType.bypass, replica_groups=RG,
                ins=[hb_in.opt()], outs=[hb_out.opt()],
            )
            return hb_out, cn_new

        def emit_hcat_load(t, X, hb_out):
            hg = state.tile([128, KT * HB], bf16, tag=f"hcat{X}", name=f"hcat{X}_{t}")
            half_rows = (KT // 2) * 128
            for piece in range(2):
                nc.scalar.dma_start(
                    hg[:, piece * 8 * HB:(piece + 1) * 8 * HB].rearrange(
                        "p (k n) -> p k n", k=8),
                    hb_out[piece * half_rows:(piece + 1) * half_rows, :].rearrange(
                        "(k p) n -> p k n", p=128),
                )
            return hg

        # ================= main loop =================
        # The AllGather result for half X is DMA-loaded during the OTHER
        # half's next block: by then the AG is (nearly) done, so the load's
        # semaphore wait doesn't head-of-line-block the scalar queue.
        pending_h = [None, None]
        for t in range(L):
            for X in range(2):
                hc = hcat[X]
                if t > 0:
                    ar_out = emit_fc_out(t, X, hc)
                O = 1 - X
                if pending_h[O] is not None:
                    hcat[O] = emit_hcat_load(t, O, pending_h[O])
                    pending_h[O] = None
                gb = emit_whh(t, X, hc)
                if t > 0:
                    ob0, ob1 = emit_ar_consume(t, X, ar_out, t - 1)
                    outb0[X], outb1[X] = ob0, ob1
                emit_tail(t, X, gb, outb0[X], outb1[X])
                hb_out, cn = emit_lstm_ag(t, X, gb)
                c_prev[X] = cn
                pending_h[X] = hb_out

        # ================= epilogue: out(L-1) =================
        for X in range(2):
            O = 1 - X
            if pending_h[O] is not None:
                hcat[O] = emit_hcat_load(L + X, O, pending_h[O])
                pending_h[O] = None
            ar_out = emit_fc_out(L + X, X, hcat[X])
            emit_ar_consume(L + X, X, ar_out, L - 1)

    nc.compile()
    return nc


def _prepare_in_maps(inputs):
    bf = ml_dtypes.bfloat16
    f = {k: np.asarray(v) for k, v in inputs.items()}
    W_enc = f["W_enc"].astype(np.float32)
    b_enc = f["b_enc"].astype(np.float32)
    W_ih = f["W_ih"].astype(np.float32)
    b_ih = f["b_ih"].astype(np.float32)
    W_hh = f["W_hh"].astype(np.float32)
    b_hh = f["b_hh"].astype(np.float32)
    W_fc1 = f["W_fc1"].astype(np.float32)
    b_fc1 = f["b_fc1"].astype(np.float32)
    W_fc2 = f["W_fc2"].astype(np.float32)
    b_fc2 = f["b_fc2"].astype(np.float32)
    W_inh = f["W_inh"].astype(np.float32)
    b_inh = f["b_inh"].astype(np.float32)
    W_inc = f["W_inc"].astype(np.float32)
    b_inc = f["b_inc"].astype(np.float32)
    labels = f["labels"].astype(np.int64)
    x = f["inputs"].astype(np.float32)

    frame0 = x.reshape(B, OUT)
    h0 = frame0 @ W_inh.T + b_inh            # [B, H]
    c0 = frame0 @ W_inc.T + b_inc            # [B, H]
    onehot = np.zeros((NCLS, B), np.float32)
    onehot[labels, np.arange(B)] = 1.0
    M1 = W_ih[:, OUT:] @ W_enc               # [4H, NCLS]
    bias_gates = b_ih + b_hh + W_ih[:, OUT:] @ b_enc  # [4H]

    in_maps = []
    for j in range(NC):
        mt = np.arange(8)
        gt, g = mt % 4, mt // 4
        rows = (gt[:, None] * H + j * 256 + g[:, None] * 128
                + np.arange(128)[None, :]).reshape(-1)
        zrows = j * 256 + np.arange(256)
        bg = bias_gates[rows].reshape(8, 128).T.copy()          # [128, 8]
        bzv = b_fc1[zrows].reshape(2, 128).T.copy()             # [128, 2]
        bov = np.zeros((128, 2), np.float32)
        bov[:, 0] = b_fc2[:128] / NC
        bov[:MR[1], 1] = b_fc2[128:] / NC
        in_maps.append({
            "whh": np.ascontiguousarray(W_hh[rows].T).astype(bf),
            "wih": np.ascontiguousarray(W_ih[rows, :OUT].T).astype(bf),
            "moh": np.ascontiguousarray(M1[rows].T).astype(bf),
            "wfc1": np.ascontiguousarray(W_fc1[zrows].T).astype(bf),
            "wfc2": np.ascontiguousarray(W_fc2[:, zrows].T).astype(bf),
            "onehot": onehot.astype(bf),
            "bgates": bg,
            "bz": bzv,
            "bo": bov,
            "h0": np.ascontiguousarray(h0.T).astype(bf),
            "c0": np.ascontiguousarray(c0.T[zrows]).astype(np.float32),
            "out0": np.ascontiguousarray(frame0.T).astype(bf),
        })
    return in_maps


def _get_program(L):
    if L not in _CACHE:
        _CACHE[L] = _build(L)
    return _CACHE[L]


def kernel(**inputs):
    from concourse.bass_utils import run_bass_kernel_spmd

    L = int(np.asarray(inputs["length"]))
    x = np.asarray(inputs["inputs"])
    Bq, J, D = x.shape
    assert (Bq, J * D) == (B, OUT)

    nc = _get_program(L)
    in_maps = _prepare_in_maps(inputs)
    res = run_bass_kernel_spmd(nc, in_maps, core_ids=list(range(NC)))
    # core j returns [L, OUT, 2*BL]; cols 0:BL hold batch columns j*BL:(j+1)*BL
    full = np.concatenate([res.results[j]["outs"][:, :, :BL] for j in range(NC)],
                          axis=2)
    out = np.transpose(full, (2, 0, 1)).reshape(B, L, J, D).astype(np.float32)
    return out


# revision 14
# speedup vs baseline: 1.8711x; 1.6394x over previous
"""Trainium2 Bass kernel for nn_DecoderRNN (240-step LSTM decoder, B=512, H=2048).

Sharding: 8-way tensor parallel on the hidden/gate dims, with the batch split
into two halves (256 columns each) that are processed as two software-pipelined
streams. While half A waits on its collectives (h AllGather after the LSTM
cell, out AllReduce after fc2), the tensor engine runs half B's matmuls, so the
PE never idles long enough for the HAM clock gate to re-throttle it (the
previous version ran every matmul at the cold 1.2 GHz clock and still idled
49% of the time).

Per core: 1024 gate rows (128-row tiles of i/f/g/o x 2 groups), 256 h rows,
256 z rows. fc2 is k-sharded: each core computes W_fc2[:, own z rows] @ z_own
and an AllReduce produces out (replacing the z AllGather + replicated fc2).
Each core's h rows are globally contiguous [j*256,(j+1)*256), so the rank-major
AllGather output is already in natural hidden order - no permutation anywhere.

Block for step t, half X (hcat = gathered h(t)):
  fc1 -> relu -> fc2 partial -> AllReduce out(t-1)   (out AR in flight ...)
  W_hh k-chains -> gates(t) psum (4 banks, 2 m-tiles packed per bank)
  load AR result, cast out(t-1) to bf16, store outs[t-1]
  gates(t) += W_ih @ out(t-1) + M1 @ onehot  (tail)
  LSTM cell -> h(t+1) -> DMA -> AllGather -> load hcat for next block
Block 0 skips the fc1/out part (out(-1) = frame0); an epilogue computes
out(L-1) only.
"""

import sys

if "/opt/trn_rl_repo" not in sys.path:
    sys.path.insert(0, "/opt/trn_rl_repo")

import numpy as np
import ml_dtypes

B = 512
HB = 256        # batch columns per half
OUT = 165
H = 2048
NCLS = 40
NC = 8
BL = B // NC    # output batch columns owned per core
KT = H // 128   # 16 k-tiles over the hidden dim
MR = [128, OUT - 128]

_CACHE = {}


def _build(L):
    import concourse.bacc as bacc
    import concourse.mybir as mybir
    import concourse.tile as tile
    from concourse.bass import ds
    from contextlib import ExitStack

    f32 = mybir.dt.float32
    bf16 = mybir.dt.bfloat16
    AF = mybir.ActivationFunctionType
    RG = [list(range(NC))]

    nc = bacc.Bacc("TRN2", target_bir_lowering=False, debug=False, num_devices=NC)

    whh_d = nc.dram_tensor("whh", [H, 1024], bf16, kind="ExternalInput")
    wih_d = nc.dram_tensor("wih", [OUT, 1024], bf16, kind="ExternalInput")
    moh_d = nc.dram_tensor("moh", [NCLS, 1024], bf16, kind="ExternalInput")
    wfc1_d = nc.dram_tensor("wfc1", [H, 256], bf16, kind="ExternalInput")
    wfc2_d = nc.dram_tensor("wfc2", [256, OUT], bf16, kind="ExternalInput")
    onehot_d = nc.dram_tensor("onehot", [NCLS, B], bf16, kind="ExternalInput")
    bgates_d = nc.dram_tensor("bgates", [128, 8], f32, kind="ExternalInput")
    bz_d = nc.dram_tensor("bz", [128, 2], f32, kind="ExternalInput")
    bo_d = nc.dram_tensor("bo", [128, 2], f32, kind="ExternalInput")
    h0_d = nc.dram_tensor("h0", [H, B], bf16, kind="ExternalInput")
    c0_d = nc.dram_tensor("c0", [256, B], f32, kind="ExternalInput")
    out0_d = nc.dram_tensor("out0", [OUT, B], bf16, kind="ExternalInput")
    # cols 0:BL always hold this core's batch slice; cols BL:2*BL are scratch
    # written by the other half's store (keeps the program identical per core).
    outs_d = nc.dram_tensor("outs", [L, OUT, 2 * BL], f32, kind="ExternalOutput")

    with tile.TileContext(nc) as tc, ExitStack() as ctx:
        const = ctx.enter_context(tc.tile_pool(name="const", bufs=1))
        state = ctx.enter_context(tc.tile_pool(name="state", bufs=2))
        work = ctx.enter_context(tc.tile_pool(name="work", bufs=2))
        psum = ctx.enter_context(tc.tile_pool(name="psum", bufs=8, space="PSUM"))
        dram = ctx.enter_context(tc.tile_pool(name="dram", bufs=3, space="DRAM"))

        pid = nc.gpsimd.partition_id()
        own_half = pid // 4          # which batch half holds this core's columns
        other_half = (pid // 4 + 1) % 2
        csrc = pid % 4 * BL          # column offset of our slice inside that half
        # store dst: the block whose half == own_half writes cols [0,BL)
        dst_off = [own_half * BL, other_half * BL]

        # ---- constants into SBUF
        whh_sb = const.tile([128, KT * 1024], bf16, name="whh_sb")
        nc.sync.dma_start(
            whh_sb.rearrange("p (k m) -> p k m", k=KT),
            whh_d.ap().rearrange("(k p) m -> p k m", p=128),
        )
        wih0_sb = const.tile([128, 1024], bf16, name="wih0_sb")
        nc.sync.dma_start(wih0_sb[:], wih_d.ap()[0:128, :])
        wih1_sb = const.tile([37, 1024], bf16, name="wih1_sb")
        nc.sync.dma_start(wih1_sb[:], wih_d.ap()[128:165, :])
        moh_sb = const.tile([NCLS, 1024], bf16, name="moh_sb")
        nc.sync.dma_start(moh_sb[:], moh_d.ap()[:, :])
        wfc1_sb = const.tile([128, KT * 256], bf16, name="wfc1_sb")
        nc.sync.dma_start(
            wfc1_sb.rearrange("p (k m) -> p k m", k=KT),
            wfc1_d.ap().rearrange("(k p) m -> p k m", p=128),
        )
        wfc2_sb = const.tile([128, 2 * OUT], bf16, name="wfc2_sb")
        nc.sync.dma_start(
            wfc2_sb.rearrange("p (k m) -> p k m", k=2),
            wfc2_d.ap().rearrange("(k p) m -> p k m", p=128),
        )
        onehot_sb = const.tile([NCLS, B], bf16, name="onehot_sb")
        nc.sync.dma_start(onehot_sb[:], onehot_d.ap()[:, :])
        bg_sb = const.tile([128, 8], f32, name="bg_sb")
        nc.sync.dma_start(bg_sb[:], bgates_d.ap()[:, :])
        bz_sb = const.tile([128, 2], f32, name="bz_sb")
        nc.sync.dma_start(bz_sb[:], bz_d.ap()[:, :])
        bo_sb = const.tile([128, 2], f32, name="bo_sb")
        nc.sync.dma_start(bo_sb[:], bo_d.ap()[:, :])

        # ---- initial state per half: hcat, c, outb
        hcat = [None, None]
        c_prev = [[None, None], [None, None]]
        outb0 = [None, None]
        outb1 = [None, None]
        for X in range(2):
            hg = state.tile([128, KT * HB], bf16, tag=f"hcat{X}", name=f"hcat{X}_init")
            nc.scalar.dma_start(
                hg.rearrange("p (k n) -> p k n", k=KT),
                h0_d.ap()[:, X * HB:(X + 1) * HB].rearrange("(k p) n -> p k n", p=128),
            )
            hcat[X] = hg
            for g in range(2):
                ct = state.tile([128, HB], f32, tag=f"c{X}{g}", name=f"c{X}{g}_init")
                nc.sync.dma_start(ct[:], c0_d.ap()[g * 128:(g + 1) * 128, X * HB:(X + 1) * HB])
                c_prev[X][g] = ct
            ob0 = state.tile([128, HB], bf16, tag=f"outb0{X}", name=f"outb0{X}_init")
            nc.sync.dma_start(ob0[:], out0_d.ap()[0:128, X * HB:(X + 1) * HB])
            outb0[X] = ob0
            ob1 = state.tile([37, HB], bf16, tag=f"outb1{X}", name=f"outb1{X}_init")
            nc.sync.dma_start(ob1[:], out0_d.ap()[128:165, X * HB:(X + 1) * HB])
            outb1[X] = ob1

        def emit_fc_out(t, X, hc):
            """fc1 -> relu -> fc2 partial -> AllReduce; returns ar_out dram tile.
            Produces out(t-1) for step t's tail (and outs[t-1] store)."""
            # one start/stop per bank: start clears has_written for the WHOLE
            # 2KB bank; each region's first start=False matmul overwrites
            # (bit clear) and later ones accumulate.
            ps_z = psum.tile([128, 512], f32, tag="ps", name=f"psz_{t}_{X}")
            for mt in range(2):
                for ki in range(KT):
                    nc.tensor.matmul(
                        ps_z[:, mt * HB:(mt + 1) * HB],
                        wfc1_sb[:, ki * 256 + mt * 128: ki * 256 + (mt + 1) * 128],
                        hc[:, ki * HB:(ki + 1) * HB],
                        start=(mt == 0 and ki == 0),
                        stop=(mt == 1 and ki == KT - 1),
                    )
            zb = work.tile([128, 512], bf16, tag=f"zb{X}", name=f"zb_{t}_{X}")
            for mt in range(2):
                nc.scalar.activation(
                    zb[:, mt * HB:(mt + 1) * HB], ps_z[:, mt * HB:(mt + 1) * HB],
                    AF.Relu, bias=bz_sb[:, mt:mt + 1],
                )
            ps_o = psum.tile([128, 512], f32, tag="ps", name=f"pso_{t}_{X}")
            for mt in range(2):
                mr = MR[mt]
                for ki in range(2):
                    nc.tensor.matmul(
                        ps_o[:mr, mt * HB:mt * HB + HB],
                        wfc2_sb[:, ki * OUT + mt * 128: ki * OUT + mt * 128 + mr],
                        zb[:, ki * HB:(ki + 1) * HB],
                        start=(mt == 0 and ki == 0),
                        stop=(mt == 1 and ki == 1),
                    )
            of = work.tile([128, 512], f32, tag=f"of{X}", name=f"of_{t}_{X}")
            for mt in range(2):
                mr = MR[mt]
                nc.scalar.activation(
                    of[:mr, mt * HB:mt * HB + HB], ps_o[:mr, mt * HB:mt * HB + HB],
                    AF.Identity, bias=bo_sb[:mr, mt:mt + 1],
                )
            ar_in = dram.tile([OUT, HB], f32, tag=f"arin{X}", name=f"arin_{t}_{X}")
            nc.sync.dma_start(ar_in[0:128, :], of[:, 0:HB])
            nc.sync.dma_start(ar_in[128:165, :], of[:37, HB:2 * HB])
            ar_out = dram.tile([OUT, HB], f32, tag=f"arout{X}", name=f"arout_{t}_{X}",
                               addr_space="Shared")
            nc.gpsimd.collective_compute(
                "AllReduce", mybir.AluOpType.add, replica_groups=RG,
                ins=[ar_in.opt()], outs=[ar_out.opt()],
            )
            return ar_out

        def emit_whh(t, X, hc):
            """W_hh k-chains into 4 packed psum banks: bank b = (mt 2b, 2b+1)."""
            gb = []
            for b in range(4):
                ps = psum.tile([128, 512], f32, tag="ps", name=f"psg_{t}_{X}_{b}")
                gb.append(ps)
                for sub in range(2):
                    mt = b * 2 + sub
                    for ki in range(KT):
                        nc.tensor.matmul(
                            ps[:, sub * HB:(sub + 1) * HB],
                            whh_sb[:, ki * 1024 + mt * 128: ki * 1024 + (mt + 1) * 128],
                            hc[:, ki * HB:(ki + 1) * HB],
                            start=(sub == 0 and ki == 0),
                            stop=False,
                        )
            return gb

        def emit_ar_consume(t, X, ar_out, store_t):
            """Load the AllReduce result: cast to bf16 outb tiles + store outs."""
            arl = work.tile([128, 512], f32, tag=f"arl{X}", name=f"arl_{t}_{X}")
            nc.sync.dma_start(arl[:, 0:HB], ar_out[0:128, :])
            nc.sync.dma_start(arl[:37, HB:2 * HB], ar_out[128:165, :])
            ob0 = state.tile([128, HB], bf16, tag=f"outb0{X}", name=f"outb0_{t}_{X}")
            nc.vector.tensor_copy(ob0[:], arl[:, 0:HB])
            ob1 = state.tile([37, HB], bf16, tag=f"outb1{X}", name=f"outb1_{t}_{X}")
            nc.vector.tensor_copy(ob1[:], arl[:37, HB:2 * HB])
            return ob0, ob1, arl

        def emit_outs_store(t, X, arl, store_t):
            # gpsimd-only (dynamic register offsets); emitted AFTER the
            # AllGather trigger so its AllReduce wait doesn't delay the AG.
            nc.gpsimd.dma_start(
                outs_d.ap()[store_t, 0:128, ds(dst_off[X], BL)],
                arl[:, ds(csrc, BL)],
            )
            nc.gpsimd.dma_start(
                outs_d.ap()[store_t, 128:165, ds(dst_off[X], BL)],
                arl[:37, ds(HB + csrc, BL)],
            )

        def emit_tail(t, X, gb, ob0, ob1):
            # moh matmuls first: they depend only on constants, so they give
            # the PE work while the out AllReduce (-> ob0/ob1 cast) finishes.
            for b in range(4):
                for sub in range(2):
                    mt = b * 2 + sub
                    dst = gb[b][:, sub * HB:(sub + 1) * HB]
                    nc.tensor.matmul(dst, moh_sb[:, mt * 128:(mt + 1) * 128],
                                     onehot_sb[:, X * HB:(X + 1) * HB],
                                     start=False, stop=False)
            for b in range(4):
                for sub in range(2):
                    mt = b * 2 + sub
                    dst = gb[b][:, sub * HB:(sub + 1) * HB]
                    nc.tensor.matmul(dst, wih0_sb[:, mt * 128:(mt + 1) * 128], ob0[:],
                                     start=False, stop=False)
                    nc.tensor.matmul(dst, wih1_sb[:, mt * 128:(mt + 1) * 128], ob1[:],
                                     start=False, stop=(sub == 1))

        def emit_lstm_ag(t, X, gb):
            """LSTM cell from gate banks -> h(t+1) slice -> AllGather; returns
            (hb_out dram tile, new c tiles)."""
            hn = work.tile([128, 512], bf16, tag=f"hn{X}", name=f"hn_{t}_{X}")
            cn_new = [None, None]
            for g in range(2):
                bi, bo_ = gb[2 * g], gb[2 * g + 1]
                si = work.tile([128, HB], f32, tag=f"si{X}", name=f"si_{t}_{X}_{g}")
                nc.scalar.activation(si[:], bi[:, 0:HB], AF.Sigmoid,
                                     bias=bg_sb[:, 4 * g + 0: 4 * g + 1])
                sf = work.tile([128, HB], f32, tag=f"sf{X}", name=f"sf_{t}_{X}_{g}")
                nc.scalar.activation(sf[:], bi[:, HB:2 * HB], AF.Sigmoid,
                                     bias=bg_sb[:, 4 * g + 1: 4 * g + 2])
                tg = work.tile([128, HB], f32, tag=f"tg{X}", name=f"tg_{t}_{X}_{g}")
                nc.scalar.activation(tg[:], bo_[:, 0:HB], AF.Tanh,
                                     bias=bg_sb[:, 4 * g + 2: 4 * g + 3])
                so = work.tile([128, HB], f32, tag=f"so{X}", name=f"so_{t}_{X}_{g}")
                nc.scalar.activation(so[:], bo_[:, HB:2 * HB], AF.Sigmoid,
                                     bias=bg_sb[:, 4 * g + 3: 4 * g + 4])
                m1 = work.tile([128, HB], f32, tag=f"m1{X}", name=f"m1_{t}_{X}_{g}")
                nc.vector.tensor_mul(m1[:], si[:], tg[:])
                m2 = work.tile([128, HB], f32, tag=f"m2{X}", name=f"m2_{t}_{X}_{g}")
                nc.vector.tensor_mul(m2[:], sf[:], c_prev[X][g][:])
                cn = state.tile([128, HB], f32, tag=f"c{X}{g}", name=f"c{X}{g}_{t}")
                nc.vector.tensor_add(cn[:], m1[:], m2[:])
                cn_new[g] = cn
                th = work.tile([128, HB], f32, tag=f"th{X}", name=f"th_{t}_{X}_{g}")
                nc.scalar.activation(th[:], cn[:], AF.Tanh)
                nc.vector.tensor_mul(hn[:, g * HB:(g + 1) * HB], so[:], th[:])
            hb_in = dram.tile([256, HB], bf16, tag=f"hbin{X}", name=f"hbin_{t}_{X}")
            nc.sync.dma_start(
                hb_in.rearrange("(g p) n -> p g n", p=128),
                hn.rearrange("p (g n) -> p g n", g=2),
            )
            hb_out = dram.tile([H, HB], bf16, tag=f"hbout{X}", name=f"hbout_{t}_{X}",
                               addr_space="Shared")
            nc.gpsimd.collective_compute(
                "AllGather", mybir.AluOpType.bypass, replica_groups=RG,
                ins=[hb_in.opt()], outs=[hb_out.opt()],
            )
            return hb_out, cn_new

        def emit_hcat_load(t, X, hb_out):
            # two 512KB pieces on different DMA paths (scalar HWDGE + gpsimd
            # SWDGE) so the data phases run in parallel.
            hg = state.tile([128, KT * HB], bf16, tag=f"hcat{X}", name=f"hcat{X}_{t}")
            half_rows = (KT // 2) * 128
            for piece, eng in ((0, nc.scalar), (1, nc.gpsimd)):
                eng.dma_start(
                    hg[:, piece * 8 * HB:(piece + 1) * 8 * HB].rearrange(
                        "p (k n) -> p k n", k=8),
                    hb_out[piece * half_rows:(piece + 1) * half_rows, :].rearrange(
                        "(k p) n -> p k n", p=128),
                )
            return hg

        # ================= main loop =================
        # The AllGather result for half X is DMA-loaded during the OTHER
        # half's next block: by then the AG is (nearly) done, so the load's
        # semaphore wait doesn't head-of-line-block the scalar queue.
        pending_h = [None, None]
        for t in range(L):
            for X in range(2):
                hc = hcat[X]
                if t > 0:
                    ar_out = emit_fc_out(t, X, hc)
                O = 1 - X
                if pending_h[O] is not None:
                    hcat[O] = emit_hcat_load(t, O, pending_h[O])
                    pending_h[O] = None
                gb = emit_whh(t, X, hc)
                if t > 0:
                    ob0, ob1, arl = emit_ar_consume(t, X, ar_out, t - 1)
                    outb0[X], outb1[X] = ob0, ob1
                emit_tail(t, X, gb, outb0[X], outb1[X])
                hb_out, cn = emit_lstm_ag(t, X, gb)
                c_prev[X] = cn
                pending_h[X] = hb_out
                if t > 0:
                    emit_outs_store(t, X, arl, t - 1)

        # ================= epilogue: out(L-1) =================
        for X in range(2):
            O = 1 - X
            if pending_h[O] is not None:
                hcat[O] = emit_hcat_load(L + X, O, pending_h[O])
                pending_h[O] = None
            ar_out = emit_fc_out(L + X, X, hcat[X])
            _, _, arl = emit_ar_consume(L + X, X, ar_out, L - 1)
            emit_outs_store(L + X, X, arl, L - 1)

    nc.compile()
    return nc


def _prepare_in_maps(inputs):
    bf = ml_dtypes.bfloat16
    f = {k: np.asarray(v) for k, v in inputs.items()}
    W_enc = f["W_enc"].astype(np.float32)
    b_enc = f["b_enc"].astype(np.float32)
    W_ih = f["W_ih"].astype(np.float32)
    b_ih = f["b_ih"].astype(np.float32)
    W_hh = f["W_hh"].astype(np.float32)
    b_hh = f["b_hh"].astype(np.float32)
    W_fc1 = f["W_fc1"].astype(np.float32)
    b_fc1 = f["b_fc1"].astype(np.float32)
    W_fc2 = f["W_fc2"].astype(np.float32)
    b_fc2 = f["b_fc2"].astype(np.float32)
    W_inh = f["W_inh"].astype(np.float32)
    b_inh = f["b_inh"].astype(np.float32)
    W_inc = f["W_inc"].astype(np.float32)
    b_inc = f["b_inc"].astype(np.float32)
    labels = f["labels"].astype(np.int64)
    x = f["inputs"].astype(np.float32)

    frame0 = x.reshape(B, OUT)
    h0 = frame0 @ W_inh.T + b_inh            # [B, H]
    c0 = frame0 @ W_inc.T + b_inc            # [B, H]
    onehot = np.zeros((NCLS, B), np.float32)
    onehot[labels, np.arange(B)] = 1.0
    M1 = W_ih[:, OUT:] @ W_enc               # [4H, NCLS]
    bias_gates = b_ih + b_hh + W_ih[:, OUT:] @ b_enc  # [4H]

    in_maps = []
    for j in range(NC):
        mt = np.arange(8)
        gt, g = mt % 4, mt // 4
        rows = (gt[:, None] * H + j * 256 + g[:, None] * 128
                + np.arange(128)[None, :]).reshape(-1)
        zrows = j * 256 + np.arange(256)
        bg = bias_gates[rows].reshape(8, 128).T.copy()          # [128, 8]
        bzv = b_fc1[zrows].reshape(2, 128).T.copy()             # [128, 2]
        bov = np.zeros((128, 2), np.float32)
        bov[:, 0] = b_fc2[:128] / NC
        bov[:MR[1], 1] = b_fc2[128:] / NC
        in_maps.append({
            "whh": np.ascontiguousarray(W_hh[rows].T).astype(bf),
            "wih": np.ascontiguousarray(W_ih[rows, :OUT].T).astype(bf),
            "moh": np.ascontiguousarray(M1[rows].T).astype(bf),
            "wfc1": np.ascontiguousarray(W_fc1[zrows].T).astype(bf),
            "wfc2": np.ascontiguousarray(W_fc2[:, zrows].T).astype(bf),
            "onehot": onehot.astype(bf),
            "bgates": bg,
            "bz": bzv,
            "bo": bov,
            "h0": np.ascontiguousarray(h0.T).astype(bf),
            "c0": np.ascontiguousarray(c0.T[zrows]).astype(np.float32),
            "out0": np.ascontiguousarray(frame0.T).astype(bf),
        })
    return in_maps


def _get_program(L):
    if L not in _CACHE:
        _CACHE[L] = _build(L)
    return _CACHE[L]


def kernel(**inputs):
    from concourse.bass_utils import run_bass_kernel_spmd

    L = int(np.asarray(inputs["length"]))
    x = np.asarray(inputs["inputs"])
    Bq, J, D = x.shape
    assert (Bq, J * D) == (B, OUT)

    nc = _get_program(L)
    in_maps = _prepare_in_maps(inputs)
    res = run_bass_kernel_spmd(nc, in_maps, core_ids=list(range(NC)))
    # core j returns [L, OUT, 2*BL]; cols 0:BL hold batch columns j*BL:(j+1)*BL
    full = np.concatenate([res.results[j]["outs"][:, :, :BL] for j in range(NC)],
                          axis=2)
    out = np.transpose(full, (2, 0, 1)).reshape(B, L, J, D).astype(np.float32)
    return out


# revision 17
# speedup vs baseline: 2.1051x; 1.1251x over previous
"""Trainium2 Bass kernel for nn_DecoderRNN (240-step LSTM decoder, B=512, H=2048).

Sharding: 8-way tensor parallel on the hidden/gate dims, with the batch split
into two halves (256 columns each) that are processed as two software-pipelined
streams. While half A waits on its collectives (h AllGather after the LSTM
cell, out AllReduce after fc2), the tensor engine runs half B's matmuls, so the
PE never idles long enough for the HAM clock gate to re-throttle it (the
previous version ran every matmul at the cold 1.2 GHz clock and still idled
49% of the time).

Per core: 1024 gate rows (128-row tiles of i/f/g/o x 2 groups), 256 h rows,
256 z rows. fc2 is k-sharded: each core computes W_fc2[:, own z rows] @ z_own
and an AllReduce produces out (replacing the z AllGather + replicated fc2).
Each core's h rows are globally contiguous [j*256,(j+1)*256), so the rank-major
AllGather output is already in natural hidden order - no permutation anywhere.

Block for step t, half X (hcat = gathered h(t)):
  fc1 -> relu -> fc2 partial -> AllReduce out(t-1)   (out AR in flight ...)
  W_hh k-chains -> gates(t) psum (4 banks, 2 m-tiles packed per bank,
    single start/stop per bank - start=True clears the whole 2KB bank)
  load AR result, cast out(t-1) to bf16, store outs[t-1]
  gates(t) += M1 @ onehot + W_ih @ out(t-1)  (tail; moh first since it has
    no AR dependency, wih1 last since it needs the second AR-load chunk)
  LSTM cell -> h(t+1) -> DMA -> AllGather (trigger ordered before the outs
    stores on the gpsimd queue so the AR wait doesn't delay it)
The gathered h for half X is DMA-loaded during the OTHER half's next block,
when the AllGather is nearly complete, so the load's semaphore wait doesn't
head-of-line-block the scalar queue. Block 0 skips the fc1/out part
(out(-1) = frame0); an epilogue computes out(L-1) only.

Per-step floor is set by the two chained 8-rank collectives per half
(AllReduce [165,256]f32 and AllGather [256,256]bf16 -> [2048,256], each
~24us trigger-to-done on this fabric); fp8 transport and bf16 AllReduce
were evaluated and rejected for accuracy (sim rel err 3e-2 / 1.2e-2 vs
the 2e-2 budget; bf16 everywhere keeps it at 4.1e-3).
"""

import sys

if "/opt/trn_rl_repo" not in sys.path:
    sys.path.insert(0, "/opt/trn_rl_repo")

import numpy as np
import ml_dtypes

B = 512
HB = 256        # batch columns per half
OUT = 165
H = 2048
NCLS = 40
NC = 8
BL = B // NC    # output batch columns owned per core
KT = H // 128   # 16 k-tiles over the hidden dim
MR = [128, OUT - 128]

_CACHE = {}


def _build(L):
    import concourse.bacc as bacc
    import concourse.mybir as mybir
    import concourse.tile as tile
    from concourse.bass import ds
    from contextlib import ExitStack

    f32 = mybir.dt.float32
    bf16 = mybir.dt.bfloat16
    AF = mybir.ActivationFunctionType
    RG = [list(range(NC))]

    nc = bacc.Bacc("TRN2", target_bir_lowering=False, debug=False, num_devices=NC)

    whh_d = nc.dram_tensor("whh", [H, 1024], bf16, kind="ExternalInput")
    wih_d = nc.dram_tensor("wih", [OUT, 1024], bf16, kind="ExternalInput")
    moh_d = nc.dram_tensor("moh", [NCLS, 1024], bf16, kind="ExternalInput")
    wfc1_d = nc.dram_tensor("wfc1", [H, 256], bf16, kind="ExternalInput")
    wfc2_d = nc.dram_tensor("wfc2", [256, OUT], bf16, kind="ExternalInput")
    onehot_d = nc.dram_tensor("onehot", [NCLS, B], bf16, kind="ExternalInput")
    bgates_d = nc.dram_tensor("bgates", [128, 8], f32, kind="ExternalInput")
    bz_d = nc.dram_tensor("bz", [128, 2], f32, kind="ExternalInput")
    bo_d = nc.dram_tensor("bo", [128, 2], f32, kind="ExternalInput")
    h0_d = nc.dram_tensor("h0", [H, B], bf16, kind="ExternalInput")
    c0_d = nc.dram_tensor("c0", [256, B], f32, kind="ExternalInput")
    out0_d = nc.dram_tensor("out0", [OUT, B], bf16, kind="ExternalInput")
    # cols 0:BL always hold this core's batch slice; cols BL:2*BL are scratch
    # written by the other half's store (keeps the program identical per core).
    outs_d = nc.dram_tensor("outs", [L, OUT, 2 * BL], f32, kind="ExternalOutput")

    with tile.TileContext(nc) as tc, ExitStack() as ctx:
        const = ctx.enter_context(tc.tile_pool(name="const", bufs=1))
        state = ctx.enter_context(tc.tile_pool(name="state", bufs=2))
        work = ctx.enter_context(tc.tile_pool(name="work", bufs=2))
        psum = ctx.enter_context(tc.tile_pool(name="psum", bufs=8, space="PSUM"))
        dram = ctx.enter_context(tc.tile_pool(name="dram", bufs=3, space="DRAM"))

        pid = nc.gpsimd.partition_id()
        own_half = pid // 4          # which batch half holds this core's columns
        other_half = (pid // 4 + 1) % 2
        csrc = pid % 4 * BL          # column offset of our slice inside that half
        # store dst: the block whose half == own_half writes cols [0,BL)
        dst_off = [own_half * BL, other_half * BL]

        # ---- constants into SBUF
        whh_sb = const.tile([128, KT * 1024], bf16, name="whh_sb")
        nc.sync.dma_start(
            whh_sb.rearrange("p (k m) -> p k m", k=KT),
            whh_d.ap().rearrange("(k p) m -> p k m", p=128),
        )
        wih0_sb = const.tile([128, 1024], bf16, name="wih0_sb")
        nc.sync.dma_start(wih0_sb[:], wih_d.ap()[0:128, :])
        wih1_sb = const.tile([37, 1024], bf16, name="wih1_sb")
        nc.sync.dma_start(wih1_sb[:], wih_d.ap()[128:165, :])
        moh_sb = const.tile([NCLS, 1024], bf16, name="moh_sb")
        nc.sync.dma_start(moh_sb[:], moh_d.ap()[:, :])
        wfc1_sb = const.tile([128, KT * 256], bf16, name="wfc1_sb")
        nc.sync.dma_start(
            wfc1_sb.rearrange("p (k m) -> p k m", k=KT),
            wfc1_d.ap().rearrange("(k p) m -> p k m", p=128),
        )
        wfc2_sb = const.tile([128, 2 * OUT], bf16, name="wfc2_sb")
        nc.sync.dma_start(
            wfc2_sb.rearrange("p (k m) -> p k m", k=2),
            wfc2_d.ap().rearrange("(k p) m -> p k m", p=128),
        )
        onehot_sb = const.tile([NCLS, B], bf16, name="onehot_sb")
        nc.sync.dma_start(onehot_sb[:], onehot_d.ap()[:, :])
        bg_sb = const.tile([128, 8], f32, name="bg_sb")
        nc.sync.dma_start(bg_sb[:], bgates_d.ap()[:, :])
        bz_sb = const.tile([128, 2], f32, name="bz_sb")
        nc.sync.dma_start(bz_sb[:], bz_d.ap()[:, :])
        bo_sb = const.tile([128, 2], f32, name="bo_sb")
        nc.sync.dma_start(bo_sb[:], bo_d.ap()[:, :])

        # ---- initial state per half: hcat, c, outb
        hcat = [None, None]
        c_prev = [[None, None], [None, None]]
        outb0 = [None, None]
        outb1 = [None, None]
        for X in range(2):
            hg = state.tile([128, KT * HB], bf16, tag=f"hcat{X}", name=f"hcat{X}_init")
            nc.scalar.dma_start(
                hg.rearrange("p (k n) -> p k n", k=KT),
                h0_d.ap()[:, X * HB:(X + 1) * HB].rearrange("(k p) n -> p k n", p=128),
            )
            hcat[X] = hg
            for g in range(2):
                ct = state.tile([128, HB], f32, tag=f"c{X}{g}", name=f"c{X}{g}_init")
                nc.sync.dma_start(ct[:], c0_d.ap()[g * 128:(g + 1) * 128, X * HB:(X + 1) * HB])
                c_prev[X][g] = ct
            ob0 = state.tile([128, HB], bf16, tag=f"outb0{X}", name=f"outb0{X}_init")
            nc.sync.dma_start(ob0[:], out0_d.ap()[0:128, X * HB:(X + 1) * HB])
            outb0[X] = ob0
            ob1 = state.tile([37, HB], bf16, tag=f"outb1{X}", name=f"outb1{X}_init")
            nc.sync.dma_start(ob1[:], out0_d.ap()[128:165, X * HB:(X + 1) * HB])
            outb1[X] = ob1

        def emit_fc_out(t, X, hc):
            """fc1 -> relu -> fc2 partial -> AllReduce; returns ar_out dram tile.
            Produces out(t-1) for step t's tail (and outs[t-1] store)."""
            # one start/stop per bank: start clears has_written for the WHOLE
            # 2KB bank; each region's first start=False matmul overwrites
            # (bit clear) and later ones accumulate.
            ps_z = psum.tile([128, 512], f32, tag="ps", name=f"psz_{t}_{X}")
            for mt in range(2):
                for ki in range(KT):
                    nc.tensor.matmul(
                        ps_z[:, mt * HB:(mt + 1) * HB],
                        wfc1_sb[:, ki * 256 + mt * 128: ki * 256 + (mt + 1) * 128],
                        hc[:, ki * HB:(ki + 1) * HB],
                        start=(mt == 0 and ki == 0),
                        stop=(mt == 1 and ki == KT - 1),
                    )
            zb = work.tile([128, 512], bf16, tag=f"zb{X}", name=f"zb_{t}_{X}")
            for mt in range(2):
                nc.scalar.activation(
                    zb[:, mt * HB:(mt + 1) * HB], ps_z[:, mt * HB:(mt + 1) * HB],
                    AF.Relu, bias=bz_sb[:, mt:mt + 1],
                )
            ps_o = psum.tile([128, 512], f32, tag="ps", name=f"pso_{t}_{X}")
            for mt in range(2):
                mr = MR[mt]
                for ki in range(2):
                    nc.tensor.matmul(
                        ps_o[:mr, mt * HB:mt * HB + HB],
                        wfc2_sb[:, ki * OUT + mt * 128: ki * OUT + mt * 128 + mr],
                        zb[:, ki * HB:(ki + 1) * HB],
                        start=(mt == 0 and ki == 0),
                        stop=(mt == 1 and ki == 1),
                    )
            of = work.tile([128, 512], f32, tag=f"of{X}", name=f"of_{t}_{X}")
            for mt in range(2):
                mr = MR[mt]
                nc.scalar.activation(
                    of[:mr, mt * HB:mt * HB + HB], ps_o[:mr, mt * HB:mt * HB + HB],
                    AF.Identity, bias=bo_sb[:mr, mt:mt + 1],
                )
            ar_in = dram.tile([OUT, HB], f32, tag=f"arin{X}", name=f"arin_{t}_{X}")
            nc.sync.dma_start(ar_in[0:128, :], of[:, 0:HB])
            nc.sync.dma_start(ar_in[128:165, :], of[:37, HB:2 * HB])
            ar_out = dram.tile([OUT, HB], f32, tag=f"arout{X}", name=f"arout_{t}_{X}",
                               addr_space="Shared")
            nc.gpsimd.collective_compute(
                "AllReduce", mybir.AluOpType.add, replica_groups=RG,
                ins=[ar_in.opt()], outs=[ar_out.opt()],
            )
            return ar_out

        def emit_whh(t, X, hc):
            """W_hh k-chains into 4 packed psum banks: bank b = (mt 2b, 2b+1)."""
            gb = []
            for b in range(4):
                ps = psum.tile([128, 512], f32, tag="ps", name=f"psg_{t}_{X}_{b}")
                gb.append(ps)
                for sub in range(2):
                    mt = b * 2 + sub
                    for ki in range(KT):
                        nc.tensor.matmul(
                            ps[:, sub * HB:(sub + 1) * HB],
                            whh_sb[:, ki * 1024 + mt * 128: ki * 1024 + (mt + 1) * 128],
                            hc[:, ki * HB:(ki + 1) * HB],
                            start=(sub == 0 and ki == 0),
                            stop=False,
                        )
            return gb

        def emit_ar_consume(t, X, ar_out, store_t):
            """Load the AllReduce result: cast to bf16 outb tiles + store outs."""
            arl = work.tile([128, 512], f32, tag=f"arl{X}", name=f"arl_{t}_{X}")
            nc.sync.dma_start(arl[:, 0:HB], ar_out[0:128, :])
            nc.sync.dma_start(arl[:37, HB:2 * HB], ar_out[128:165, :])
            ob0 = state.tile([128, HB], bf16, tag=f"outb0{X}", name=f"outb0_{t}_{X}")
            nc.vector.tensor_copy(ob0[:], arl[:, 0:HB])
            ob1 = state.tile([37, HB], bf16, tag=f"outb1{X}", name=f"outb1_{t}_{X}")
            nc.vector.tensor_copy(ob1[:], arl[:37, HB:2 * HB])
            return ob0, ob1, arl

        def emit_outs_store(t, X, arl, store_t):
            # gpsimd-only (dynamic register offsets); emitted AFTER the
            # AllGather trigger so its AllReduce wait doesn't delay the AG.
            nc.gpsimd.dma_start(
                outs_d.ap()[store_t, 0:128, ds(dst_off[X], BL)],
                arl[:, ds(csrc, BL)],
            )
            nc.gpsimd.dma_start(
                outs_d.ap()[store_t, 128:165, ds(dst_off[X], BL)],
                arl[:37, ds(HB + csrc, BL)],
            )

        def emit_tail(t, X, gb, ob0, ob1):
            # moh matmuls first: they depend only on constants, so they give
            # the PE work while the out AllReduce (-> ob0/ob1 cast) finishes.
            for b in range(4):
                for sub in range(2):
                    mt = b * 2 + sub
                    dst = gb[b][:, sub * HB:(sub + 1) * HB]
                    nc.tensor.matmul(dst, moh_sb[:, mt * 128:(mt + 1) * 128],
                                     onehot_sb[:, X * HB:(X + 1) * HB],
                                     start=False, stop=False)
            # wih0 needs only the first AR-load chunk (rows 0:128 -> ob0);
            # wih1 (ob1, second chunk) last, carrying the per-bank stop.
            for b in range(4):
                for sub in range(2):
                    mt = b * 2 + sub
                    dst = gb[b][:, sub * HB:(sub + 1) * HB]
                    nc.tensor.matmul(dst, wih0_sb[:, mt * 128:(mt + 1) * 128], ob0[:],
                                     start=False, stop=False)
            for b in range(4):
                for sub in range(2):
                    mt = b * 2 + sub
                    dst = gb[b][:, sub * HB:(sub + 1) * HB]
                    nc.tensor.matmul(dst, wih1_sb[:, mt * 128:(mt + 1) * 128], ob1[:],
                                     start=False, stop=(sub == 1))

        def emit_lstm_ag(t, X, gb):
            """LSTM cell from gate banks -> h(t+1) slice -> AllGather; returns
            (hb_out dram tile, new c tiles)."""
            hn = work.tile([128, 512], bf16, tag=f"hn{X}", name=f"hn_{t}_{X}")
            cn_new = [None, None]
            for g in range(2):
                bi, bo_ = gb[2 * g], gb[2 * g + 1]
                si = work.tile([128, HB], f32, tag=f"si{X}", name=f"si_{t}_{X}_{g}")
                nc.scalar.activation(si[:], bi[:, 0:HB], AF.Sigmoid,
                                     bias=bg_sb[:, 4 * g + 0: 4 * g + 1])
                sf = work.tile([128, HB], f32, tag=f"sf{X}", name=f"sf_{t}_{X}_{g}")
                nc.scalar.activation(sf[:], bi[:, HB:2 * HB], AF.Sigmoid,
                                     bias=bg_sb[:, 4 * g + 1: 4 * g + 2])
                tg = work.tile([128, HB], f32, tag=f"tg{X}", name=f"tg_{t}_{X}_{g}")
                nc.scalar.activation(tg[:], bo_[:, 0:HB], AF.Tanh,
                                     bias=bg_sb[:, 4 * g + 2: 4 * g + 3])
                so = work.tile([128, HB], f32, tag=f"so{X}", name=f"so_{t}_{X}_{g}")
                nc.scalar.activation(so[:], bo_[:, HB:2 * HB], AF.Sigmoid,
                                     bias=bg_sb[:, 4 * g + 3: 4 * g + 4])
                m1 = work.tile([128, HB], f32, tag=f"m1{X}", name=f"m1_{t}_{X}_{g}")
                nc.vector.tensor_mul(m1[:], si[:], tg[:])
                m2 = work.tile([128, HB], f32, tag=f"m2{X}", name=f"m2_{t}_{X}_{g}")
                nc.vector.tensor_mul(m2[:], sf[:], c_prev[X][g][:])
                cn = state.tile([128, HB], f32, tag=f"c{X}{g}", name=f"c{X}{g}_{t}")
                nc.vector.tensor_add(cn[:], m1[:], m2[:])
                cn_new[g] = cn
                th = work.tile([128, HB], f32, tag=f"th{X}", name=f"th_{t}_{X}_{g}")
                nc.scalar.activation(th[:], cn[:], AF.Tanh)
                nc.vector.tensor_mul(hn[:, g * HB:(g + 1) * HB], so[:], th[:])
            hb_in = dram.tile([256, HB], bf16, tag=f"hbin{X}", name=f"hbin_{t}_{X}")
            nc.sync.dma_start(
                hb_in.rearrange("(g p) n -> p g n", p=128),
                hn.rearrange("p (g n) -> p g n", g=2),
            )
            hb_out = dram.tile([H, HB], bf16, tag=f"hbout{X}", name=f"hbout_{t}_{X}",
                               addr_space="Shared")
            nc.gpsimd.collective_compute(
                "AllGather", mybir.AluOpType.bypass, replica_groups=RG,
                ins=[hb_in.opt()], outs=[hb_out.opt()],
            )
            return hb_out, cn_new

        def emit_hcat_load(t, X, hb_out):
            hg = state.tile([128, KT * HB], bf16, tag=f"hcat{X}", name=f"hcat{X}_{t}")
            half_rows = (KT // 2) * 128
            for piece, eng in ((0, nc.scalar), (1, nc.scalar)):
                eng.dma_start(
                    hg[:, piece * 8 * HB:(piece + 1) * 8 * HB].rearrange(
                        "p (k n) -> p k n", k=8),
                    hb_out[piece * half_rows:(piece + 1) * half_rows, :].rearrange(
                        "(k p) n -> p k n", p=128),
                )
            return hg

        # ================= main loop =================
        # The AllGather result for half X is DMA-loaded during the OTHER
        # half's next block: by then the AG is (nearly) done, so the load's
        # semaphore wait doesn't head-of-line-block the scalar queue.
        pending_h = [None, None]
        for t in range(L):
            for X in range(2):
                hc = hcat[X]
                if t > 0:
                    ar_out = emit_fc_out(t, X, hc)
                O = 1 - X
                if pending_h[O] is not None:
                    hcat[O] = emit_hcat_load(t, O, pending_h[O])
                    pending_h[O] = None
                gb = emit_whh(t, X, hc)
                if t > 0:
                    ob0, ob1, arl = emit_ar_consume(t, X, ar_out, t - 1)
                    outb0[X], outb1[X] = ob0, ob1
                emit_tail(t, X, gb, outb0[X], outb1[X])
                hb_out, cn = emit_lstm_ag(t, X, gb)
                c_prev[X] = cn
                pending_h[X] = hb_out
                if t > 0:
                    emit_outs_store(t, X, arl, t - 1)

        # ================= epilogue: out(L-1) =================
        for X in range(2):
            O = 1 - X
            if pending_h[O] is not None:
                hcat[O] = emit_hcat_load(L + X, O, pending_h[O])
                pending_h[O] = None
            ar_out = emit_fc_out(L + X, X, hcat[X])
            _, _, arl = emit_ar_consume(L + X, X, ar_out, L - 1)
            emit_outs_store(L + X, X, arl, L - 1)

    nc.compile()
    return nc


def _prepare_in_maps(inputs):
    bf = ml_dtypes.bfloat16
    f = {k: np.asarray(v) for k, v in inputs.items()}
    W_enc = f["W_enc"].astype(np.float32)
    b_enc = f["b_enc"].astype(np.float32)
    W_ih = f["W_ih"].astype(np.float32)
    b_ih = f["b_ih"].astype(np.float32)
    W_hh = f["W_hh"].astype(np.float32)
    b_hh = f["b_hh"].astype(np.float32)
    W_fc1 = f["W_fc1"].astype(np.float32)
    b_fc1 = f["b_fc1"].astype(np.float32)
    W_fc2 = f["W_fc2"].astype(np.float32)
    b_fc2 = f["b_fc2"].astype(np.float32)
    W_inh = f["W_inh"].astype(np.float32)
    b_inh = f["b_inh"].astype(np.float32)
    W_inc = f["W_inc"].astype(np.float32)
    b_inc = f["b_inc"].astype(np.float32)
    labels = f["labels"].astype(np.int64)
    x = f["inputs"].astype(np.float32)

    frame0 = x.reshape(B, OUT)
    h0 = frame0 @ W_inh.T + b_inh            # [B, H]
    c0 = frame0 @ W_inc.T + b_inc            # [B, H]
    onehot = np.zeros((NCLS, B), np.float32)
    onehot[labels, np.arange(B)] = 1.0
    M1 = W_ih[:, OUT:] @ W_enc               # [4H, NCLS]
    bias_gates = b_ih + b_hh + W_ih[:, OUT:] @ b_enc  # [4H]

    in_maps = []
    for j in range(NC):
        mt = np.arange(8)
        gt, g = mt % 4, mt // 4
        rows = (gt[:, None] * H + j * 256 + g[:, None] * 128
                + np.arange(128)[None, :]).reshape(-1)
        zrows = j * 256 + np.arange(256)
        bg = bias_gates[rows].reshape(8, 128).T.copy()          # [128, 8]
        bzv = b_fc1[zrows].reshape(2, 128).T.copy()             # [128, 2]
        bov = np.zeros((128, 2), np.float32)
        bov[:, 0] = b_fc2[:128] / NC
        bov[:MR[1], 1] = b_fc2[128:] / NC
        in_maps.append({
            "whh": np.ascontiguousarray(W_hh[rows].T).astype(bf),
            "wih": np.ascontiguousarray(W_ih[rows, :OUT].T).astype(bf),
            "moh": np.ascontiguousarray(M1[rows].T).astype(bf),
            "wfc1": np.ascontiguousarray(W_fc1[zrows].T).astype(bf),
            "wfc2": np.ascontiguousarray(W_fc2[:, zrows].T).astype(bf),
            "onehot": onehot.astype(bf),
            "bgates": bg,
            "bz": bzv,
            "bo": bov,
            "h0": np.ascontiguousarray(h0.T).astype(bf),
            "c0": np.ascontiguousarray(c0.T[zrows]).astype(np.float32),
            "out0": np.ascontiguousarray(frame0.T).astype(bf),
        })
    return in_maps


def _get_program(L):
    if L not in _CACHE:
        _CACHE[L] = _build(L)
    return _CACHE[L]


def kernel(**inputs):
    from concourse.bass_utils import run_bass_kernel_spmd

    L = int(np.asarray(inputs["length"]))
    x = np.asarray(inputs["inputs"])
    Bq, J, D = x.shape
    assert (Bq, J * D) == (B, OUT)

    nc = _get_program(L)
    in_maps = _prepare_in_maps(inputs)
    res = run_bass_kernel_spmd(nc, in_maps, core_ids=list(range(NC)))
    # core j returns [L, OUT, 2*BL]; cols 0:BL hold batch columns j*BL:(j+1)*BL
    full = np.concatenate([res.results[j]["outs"][:, :, :BL] for j in range(NC)],
                          axis=2)
    out = np.transpose(full, (2, 0, 1)).reshape(B, L, J, D).astype(np.float32)
    return out
